# revision 1
# baseline (speedup 1.0000x reference)
"""Sparse attention kernel for Trainium2 (8 NeuronCores, data-parallel over batch).

Reference computation (per batch row b):
    q    = x @ q_w.T                                  [N, C]
    xkv  = x[key_ind]                                 [NKV, C]
    kv   = xkv @ kv_w.T -> per-head k, v              [NKV, 2C]
    attn = softmax((q*scale) @ k.T) @ v               [N, C]
    out  = attn @ proj_w.T + proj_b                   [N, C]

Per-core layout (core = one batch row), all SBUF data bf16, PSUM f32:
  - Everything transposed ("feature on partition"): qT [C, N] per head pair,
    kT [pair, NKV], scores ST [keys, tokens] so softmax needs no transposes;
    denominator via an all-ones pre-fill of the augmented v (65th column).
  - One PSUM pool, 8 banks: st 2x[128,1024] (two score tiles share one exp),
    ov 2x[128,512], mm 2x[128,512] (kv/q/proj accumulators share a tag).
  - KV gather via indirect SWDGE DMA + PE transposes (bf16: 1 cyc/row).
  - Engine balance: exp on Act, kv-phase evacs on Act, qt/recip/at-mul on
    DVE, broadcast + bias-add on Pool, loads/stores on SP — PE never waits.
"""
import os
import sys

sys.path.insert(0, "/opt/trn_rl_repo")

import numpy as np  # noqa: E402
import ml_dtypes  # noqa: E402

B, N, C = 8, 2048, 768
NKV = 512
H = 12
HD = C // H          # 64
SCALE = HD ** -0.5
P = 128
CT = C // P          # 6 feature tiles
NC2 = 512            # token chunk
NCH = N // NC2       # 4 chunks
MCH = NKV // P       # 4 key chunks
G = H // 2           # 6 head pairs

_CACHE = {}


def _build():
    import concourse.bass as bass
    import concourse.mybir as mybir
    import concourse.tile as tile
    from concourse import bacc
    from concourse.masks import make_identity
    from contextlib import ExitStack

    F32 = mybir.dt.float32
    BF16 = mybir.dt.bfloat16
    I16 = mybir.dt.int16
    Exp = mybir.ActivationFunctionType.Exp
    Copy = mybir.ActivationFunctionType.Copy

    nc = bacc.Bacc("TRN2", target_bir_lowering=False, debug=False, num_devices=8)

    xT = nc.dram_tensor("xT", [NCH, P, CT * NC2], BF16, kind="ExternalInput")
    xr = nc.dram_tensor("xr", [N, C], BF16, kind="ExternalInput")
    idx = nc.dram_tensor("idx", [P, NKV // 16], I16, kind="ExternalInput")
    wq = nc.dram_tensor("wq", [G, P, CT * P], BF16, kind="ExternalInput")
    wk = nc.dram_tensor("wk", [P, CT * C], BF16, kind="ExternalInput")
    wv = nc.dram_tensor("wv", [P, CT * C], BF16, kind="ExternalInput")
    wp = nc.dram_tensor("wp", [P, CT * C], BF16, kind="ExternalInput")
    pb = nc.dram_tensor("pb", [P, CT], F32, kind="ExternalInput")
    out = nc.dram_tensor("out", [P, CT, NCH, NC2], BF16, kind="ExternalOutput")

    with tile.TileContext(nc) as tc, ExitStack() as top:
        const = top.enter_context(tc.tile_pool(name="const", bufs=1))
        qtp = top.enter_context(tc.tile_pool(name="qtp", bufs=4))
        ptp = top.enter_context(tc.tile_pool(name="ptp", bufs=10))
        att = top.enter_context(tc.tile_pool(name="att", bufs=3))
        rcp = top.enter_context(tc.tile_pool(name="rcp", bufs=10))
        ojp = top.enter_context(tc.tile_pool(name="ojp", bufs=8))
        ps = top.enter_context(tc.tile_pool(name="ps", bufs=1, space="PSUM"))

        # ---------- input DMAs (order = DMA-engine priority) ----------
        xTc_sb = []
        for ch in range(NCH):
            xTc_sb.append(const.tile([P, CT * NC2], BF16, tag=f"xT{ch}",
                                     name=f"xTc{ch}"))
        nc.sync.dma_start(xTc_sb[0][:], xT[0, :, :])
        idx_sb = const.tile([P, NKV // 16], I16, tag="idx")
        nc.sync.dma_start(idx_sb[:], idx[:])
        wq_sb = []
        for g in range(G):
            t = const.tile([P, CT * P], BF16, tag=f"wq{g}", name=f"wq{g}")
            nc.sync.dma_start(t[:], wq[g, :, :])
            wq_sb.append(t)

        # p-state warm-up source tile first (Pool), then vaug ones pre-fill
        warm = const.tile([P, NC2], BF16, tag="warm")
        nc.gpsimd.memset(warm[:], 0.0)
        vaug_sb = []
        for k in range(MCH):
            va = const.tile([P, H * (HD + 1)], BF16, tag=f"vaug{k}",
                            name=f"vaug{k}")
            nc.gpsimd.memset(va[:], 1.0)
            vaug_sb.append(va)

        # transposing gather: xkvT3[p, i, j] = xr[key_ind[j], i*128 + p]
        xkvT = const.tile([P, CT * NKV], BF16, tag="xkvT")
        xkvT3 = xkvT[:].rearrange("p (i m) -> p i m", i=CT)
        nc.gpsimd.dma_gather(
            out_ap=xkvT3, in_ap=xr[:], idxs_ap=idx_sb[:], num_idxs=NKV,
            num_idxs_reg=NKV, elem_size=C, transpose=True)

        wk_sb = const.tile([P, CT * C], BF16, tag="wk")
        nc.sync.dma_start(wk_sb[:], wk[:])
        wv_sb = const.tile([P, CT * C], BF16, tag="wv")
        nc.sync.dma_start(wv_sb[:], wv[:])
        pb_sb = const.tile([P, CT], F32, tag="pb")
        nc.sync.dma_start(pb_sb[:], pb[:])
        for ch in range(1, NCH):
            nc.sync.dma_start(xTc_sb[ch][:], xT[ch, :, :])
        wp_sb = const.tile([P, CT * C], BF16, tag="wp")
        nc.sync.dma_start(wp_sb[:], wp[:])

        # ---------- KV phase: q(ch0) + k/v projections ----------
        def emit_qp(ch, g):
            qp = ps.tile([P, NC2], F32, tag="mm", bufs=2, name=f"qp{ch}_{g}")
            for i in range(CT):
                nc.tensor.matmul(qp[:], wq_sb[g][:, i * P:(i + 1) * P],
                                 xTc_sb[ch][:, i * NC2:(i + 1) * NC2],
                                 start=(i == 0), stop=(i == CT - 1))
            qt = qtp.tile([P, NC2], BF16, tag=f"qT{g}", name=f"qt{ch}_{g}")
            nc.scalar.activation(qt[:], qp[:], Copy)
            return qt

        # kT per head pair g -> [128, NKV] (rows 0-63 head 2g, 64-127 head 2g+1)
        kT_sb = const.tile([P, G * NKV], BF16, tag="kT")

        def emit_kt(g):
            kp = ps.tile([P, NKV], F32, tag="mm", bufs=2, name=f"kp{g}")
            for i in range(CT):
                nc.tensor.matmul(kp[:], wk_sb[:, i * C + g * P:i * C + (g + 1) * P],
                                 xkvT3[:, i, :], start=(i == 0), stop=(i == CT - 1))
            nc.vector.tensor_copy(kT_sb[:, g * NKV:(g + 1) * NKV], kp[:])

        def emit_vaug(k):
            # v (+ ones col): [128 keys, H*(HD+1)], col HD of each head block
            # stays 1.0 from the memset pre-fill
            va3 = vaug_sb[k][:].rearrange("p (h x) -> p h x", x=HD + 1)
            for half in range(2):
                vp = ps.tile([P, CT * HD], F32, tag="mm", bufs=2,
                             name=f"vp{k}_{half}")
                for i in range(CT):
                    nc.tensor.matmul(vp[:], xkvT3[:, i, k * P:(k + 1) * P],
                                     wv_sb[:, i * C + half * CT * HD:
                                           i * C + (half + 1) * CT * HD],
                                     start=(i == 0), stop=(i == CT - 1))
                nc.vector.tensor_copy(va3[:, CT * half:CT * half + CT, 0:HD],
                                      vp[:].rearrange("p (h x) -> p h x", x=HD))

        # p-state warm-up: cheap K=4 matmuls keep the PE continuously busy
        # until the first real inputs land, so real work starts at full clock
        for w in range(21):
            d = ps.tile([P, NC2], F32, tag="mm", bufs=2, name=f"warm{w}")
            nc.tensor.matmul(d[:], warm[0:4, 0:P], warm[0:4, :],
                             start=True, stop=True)

        # q for chunk 0 (fills PE while wk/wv/gather land), then the first two
        # kT pairs and all of vaug; kT(2..5) interleave into chunk 0's loop
        qT = [emit_qp(0, g) for g in range(G)]
        emit_kt(0)
        emit_kt(1)
        for k in range(MCH):
            emit_vaug(k)

        # ---------- main loop over token chunks ----------
        def emit_oj(pp, ch, j):
            # bias-add on DVE (keeps Act free for the exp stream)
            oj = ojp.tile([P, NC2], BF16, tag="oj", name=f"oj{ch}_{j}")
            nc.vector.tensor_scalar_add(oj[:], pp[:], pb_sb[:, j:j + 1])
            nc.sync.dma_start(out[:, j, ch, :], oj[:])

        def emit_proj_one(attn, ch, j):
            pp = ps.tile([P, NC2], F32, tag="mm", bufs=2, name=f"pp{ch}_{j}")
            for i in range(CT):
                nc.tensor.matmul(
                    pp[:], wp_sb[:, i * C + j * P:i * C + (j + 1) * P],
                    attn[i][:], start=(i == 0), stop=(i == CT - 1))
            emit_oj(pp, ch, j)

        def wp_step(pp, i, j, start):
            nc.tensor.matmul(
                pp[:], wp_sb[:, i * C + j * P:i * C + (j + 1) * P],
                attn[i][:], start=start, stop=False, skip_group_check=True)

        pend_proj = None
        for ch in range(NCH):
            attn = []
            qT_next = []
            partial = {}       # last chunk: j -> incremental proj accumulator
            for g in range(G):
                at = att.tile([P, NC2], BF16, tag=f"attn{g}")
                for par in range(2):
                    h = 2 * g + par
                    base = par * HD
                    pts = []
                    for half in range(2):
                        st2 = ps.tile([P, 2 * NC2], F32, tag="st", bufs=2)
                        for hh in range(2):
                            k = 2 * half + hh
                            nc.tensor.matmul(
                                st2[:, hh * NC2:(hh + 1) * NC2],
                                kT_sb[base:base + HD,
                                      g * NKV + k * P:g * NKV + (k + 1) * P],
                                qT[g][base:base + HD, :],
                                start=True, stop=True)
                        pt = ptp.tile([P, 2 * NC2], BF16, tag="pt")
                        nc.scalar.activation(pt[:], st2[:], Exp, scale=SCALE)
                        pts.append(pt)
                    # previous chunk's proj spreads across this chunk's slots,
                    # overlapping the exp latencies (double rate on the last
                    # chunk, whose q-interleave slots are otherwise empty)
                    if pend_proj is not None:
                        if ch + 1 == NCH:
                            j = 2 * g + par
                            if j < CT:
                                emit_proj_one(pend_proj[0], pend_proj[1], j)
                            if j >= CT - 1:
                                pend_proj = None
                        elif par == 1:
                            emit_proj_one(pend_proj[0], pend_proj[1], g)
                            if g == G - 1:
                                pend_proj = None
                    # deferred kT pairs (chunk 0) and the next chunk's q
                    # projection overlap the exp latency
                    if par == 0 and ch == 0 and g + 2 < G:
                        emit_kt(g + 2)
                    if par == 0 and ch + 1 < NCH:
                        qT_next.append(emit_qp(ch + 1, g))
                    if par == 0 and ch + 1 == NCH and g >= 3:
                        # last chunk: start two incremental proj accumulators
                        # in the mm slots the q-interleave vacated (stay one
                        # head behind the attn chain to avoid stalling on it)
                        for j in range(2):
                            if j not in partial:
                                pp = ps.tile([P, NC2], F32, tag="mm", bufs=2,
                                             name=f"ppl{j}")
                                wp_step(pp, 0, j, True)
                                partial[j] = (pp, 1)
                            pp, ni = partial[j]
                            while ni < g - 1:
                                wp_step(pp, ni, j, False)
                                ni += 1
                            partial[j] = (pp, ni)
                    ov = ps.tile([HD + 1, NC2], F32, tag="ov", bufs=2)
                    for half in range(2):
                        for hh in range(2):
                            k = 2 * half + hh
                            nc.tensor.matmul(
                                ov[:], vaug_sb[k][:, h * (HD + 1):(h + 1) * (HD + 1)],
                                pts[half][:, hh * NC2:(hh + 1) * NC2],
                                start=(k == 0), stop=(k == MCH - 1))
                    rc = rcp.tile([1, NC2], F32, tag="rc")
                    nc.vector.reciprocal(rc[:], ov[HD:HD + 1, :])
                    rb = rcp.tile([HD, NC2], F32, tag="rb")
                    nc.gpsimd.partition_broadcast(rb[:], rc[:])
                    nc.vector.tensor_mul(at[base:base + HD, :], ov[0:HD, :], rb[:])
                attn.append(at)
            if ch + 1 < NCH:
                pend_proj = (attn, ch)
                qT = qT_next
            else:
                # last chunk: four more accumulators in the freed st halves,
                # catch everyone up through head pair 4, then the finals
                st_a = ps.tile([P, 2 * NC2], F32, tag="st", bufs=2, name="ppl_a")
                st_b = ps.tile([P, 2 * NC2], F32, tag="st", bufs=2, name="ppl_b")
                for j in range(2, CT):
                    half = (j - 2) % 2
                    src = st_a if j < 4 else st_b
                    pp = src[:, half * NC2:(half + 1) * NC2]
                    wp_step(pp, 0, j, True)
                    partial[j] = (pp, 1)
                for j in range(CT):
                    pp, ni = partial[j]
                    while ni < G - 1:
                        wp_step(pp, ni, j, False)
                        ni += 1
                    partial[j] = (pp, ni)
                # all finals before any oj reads a shared tile; ojs write one
                # wide tile (Act/DVE alternating) flushed by a single store
                for j in range(CT):
                    pp, ni = partial[j]
                    assert ni == G - 1
                    nc.tensor.matmul(
                        pp[:], wp_sb[:, 5 * C + j * P:5 * C + (j + 1) * P],
                        attn[5][:], start=False, stop=True, skip_group_check=True)
                ojL = ojp.tile([P, CT * NC2], BF16, tag="ojL", bufs=1, name="ojL")
                for j in range(CT):
                    pp = partial[j][0]
                    sl = ojL[:, j * NC2:(j + 1) * NC2]
                    if j % 2 == 0:
                        nc.scalar.activation(
                            sl, pp[:], mybir.ActivationFunctionType.Identity,
                            bias=pb_sb[:, j:j + 1])
                    else:
                        nc.vector.tensor_scalar_add(sl, pp[:], pb_sb[:, j:j + 1])
                ojL3 = ojL[:].rearrange("p (j n) -> p j n", j=CT)
                for q in range(3):
                    nc.sync.dma_start(out[:, 2 * q:2 * q + 2, ch, :],
                                      ojL3[:, 2 * q:2 * q + 2, :])

    nc.compile()
    return nc


def _get_nc():
    if "nc" not in _CACHE:
        _CACHE["nc"] = _build()
    return _CACHE["nc"]


def _prep_core_inputs(x, key_ind, q_w, kv_w, proj_w, proj_b):
    """Build the 8 per-core input maps (everything bf16 except idx/pb)."""
    bf16 = ml_dtypes.bfloat16

    def wT_pack(w):
        # [C(out), C(in)] weight -> transposed blocks [P, CT*C] bf16
        return np.ascontiguousarray(
            w.T.astype(np.float32).reshape(CT, P, C).transpose(1, 0, 2)
            .reshape(P, CT * C)).astype(bf16)

    # wq repacked per head pair: [G, P, CT*128]
    wq = np.ascontiguousarray(
        wT_pack(q_w).reshape(P, CT, G, P).transpose(2, 0, 1, 3)
        .reshape(G, P, CT * P))
    kvwT3 = kv_w.T.astype(np.float32).reshape(C, H, 2 * HD)
    wk = wT_pack(np.ascontiguousarray(kvwT3[:, :, :HD].reshape(C, C)).T)
    wv = wT_pack(np.ascontiguousarray(kvwT3[:, :, HD:].reshape(C, C)).T)
    wp = wT_pack(proj_w)
    pbp = np.ascontiguousarray(proj_b.astype(np.float32).reshape(CT, P).T)
    x = np.asarray(x, dtype=np.float32)
    in_maps = []
    for b in range(B):
        xb = x[b]                                   # [N, C]
        xTb = np.ascontiguousarray(
            xb.T.reshape(CT, P, NCH, NC2).transpose(2, 1, 0, 3)
            .reshape(NCH, P, CT * NC2)).astype(bf16)
        # int16 indices, index j at [j % 16, j // 16], replicated to all 128
        # partitions (16-partition wrap; walrus reads its own replica)
        idxb = np.ascontiguousarray(np.tile(
            np.asarray(key_ind[b]).astype(np.int16).reshape(NKV // 16, 16).T, (8, 1)))
        in_maps.append({
            "xT": xTb, "xr": xb.astype(bf16), "idx": idxb,
            "wq": wq, "wk": wk, "wv": wv, "wp": wp, "pb": pbp,
        })
    return in_maps


def kernel(x, key_ind, q_w, kv_w, proj_w, proj_b, _trace=False, _results=None):
    from concourse.bass_utils import run_bass_kernel_spmd

    nc = _get_nc()
    in_maps = _prep_core_inputs(x, key_ind, q_w, kv_w, proj_w, proj_b)
    res = run_bass_kernel_spmd(nc, in_maps, core_ids=list(range(B)), trace=_trace)
    if _results is not None:
        _results.append(res)
    outp = np.empty((B, N, C), dtype=np.float32)
    for b in range(B):
        o = res.results[b]["out"].astype(np.float32)   # [P, CT, NCH, NC2]
        outp[b] = o.transpose(2, 3, 1, 0).reshape(N, C)
    return outp

